# revision 15
# baseline (speedup 1.0000x reference)
"""GCN (2-layer, PyG GCNConv semantics) on 8 Trainium2 NeuronCores.

Strategy (sharding_hint: shard nodes across cores, partition edges by dst):
  - Nodes sharded contiguously: core c owns dst rows [c*NP, (c+1)*NP).
  - Hidden tables (h1' = dis*x@W1, h2' = dis*relu(z1)@W2) are AllGathered
    in 3 pieces so gathers can start early and overlap the collectives.
  - The symmetric norm dis[s]*dis[d] is factored: table rows pre-scaled by
    dis[v]; window PSUM scaled by dis[d] on the way out (ACT scale); bias
    injected as (b/dis[d]) via the pass-0 identity matmul.
  - Aggregation runs PASS-BASED: pass p processes half-p edges (src in
    AllGather piece p) of every dst window in order, so the gather queue
    never stalls on a not-yet-landed table piece.  Between passes the
    partial window sum is spilled from PSUM to SBUF (fp16, ACT copy) and
    re-injected as the rhs of the next pass's identity matmul.
  - Per (window, half): dma_gather pulls the real edge rows (trailing pad
    indices are -1 -> skipped by HW), a 0/1 mask G[e,d]=(dst_rel[e]==d)
    is built with one DVE tensor_tensor, TensorE accumulates G.T @ msg.
  - All cores run one identical program (chunk counts are the max over
    cores; pad tokens have idx=-1, dst_rel=255).
"""

import math

import numpy as np

M = 8  # cores
P = 128  # partitions
WIN_SPLIT = [16, 20, 13]  # windows per AllGather piece (=gather halves)
NH = len(WIN_SPLIT)
GCAP = 8  # max chunks (of 128 tokens) per dma_gather call (single packet)
AG2_SKEW = 8  # windows between a piece's last window and its AG trigger


def _prep(x, W1, b1, W2, b2, edge_index):
    """Host-side sharding/layout (index manipulation + dtype casts only)."""
    N, IN = x.shape
    HID = W1.shape[1]
    OUT = W2.shape[1]
    OUTP = P
    assert N % M == 0
    NP = N // M
    NPAD = math.ceil(NP / P) * P
    NT = NPAD // P
    KT = IN // P
    assert sum(WIN_SPLIT) == NT

    src = np.asarray(edge_index[0], dtype=np.int64)
    dst = np.asarray(edge_index[1], dtype=np.int64)
    deg = (np.bincount(dst, minlength=N) + 1).astype(np.float32)
    dis = 1.0 / np.sqrt(deg)

    bound = np.cumsum([0] + WIN_SPLIT)  # piece window boundaries
    piece_rows = [wn * P for wn in WIN_SPLIT]  # per-core rows per piece
    piece_of_win = np.zeros(NT, dtype=np.int64)
    for j in range(NH):
        piece_of_win[bound[j] : bound[j + 1]] = j

    sc, sl = src // NP, src % NP
    swin = sl // P
    pj = piece_of_win[swin]
    lidx = (
        sc * np.array(piece_rows)[pj]
        + (sl - bound[pj] * P)
    )
    assert int(lidx.max(initial=0)) < 32768, "gather idx must fit int16"

    owner = dst // NP
    ldst = dst - owner * NP
    win = ldst // P

    gid = (owner * NT + win) * NH + pj
    order = np.lexsort((lidx, gid))
    lidx_s = lidx[order]
    drel_s = (ldst[order] % P).astype(np.float16)

    ngroups = M * NT * NH
    counts = np.bincount(gid[order], minlength=ngroups).reshape(M, NT, NH)
    # per-(window, half) chunk count: max over cores (identical SPMD program)
    CH_wh = np.ceil(counts.max(axis=0) / P).astype(np.int64)  # [NT, NH]
    CHMAX = max(1, int(CH_wh.max()))

    # token slots / chunk columns: w-major, h inner
    slot_base = np.zeros((NT, NH), dtype=np.int64)
    col_of = np.zeros((NT, NH), dtype=np.int64)
    tb = 0
    cb = 0
    for w in range(NT):
        for h in range(NH):
            slot_base[w, h] = tb
            col_of[w, h] = cb
            tb += int(CH_wh[w, h]) * P
            cb += int(CH_wh[w, h])
    TOK = int(tb)
    NCHUNK = int(cb)

    group_starts = np.zeros(ngroups + 1, dtype=np.int64)
    np.cumsum(counts.reshape(-1), out=group_starts[1:])

    in_maps = []
    f16 = np.float16
    w1f = np.ascontiguousarray(W1, dtype=f16)
    b1f = np.ascontiguousarray(
        np.broadcast_to(np.asarray(b1, dtype=f16).reshape(1, HID), (P, HID))
    )
    w2f = np.zeros((HID, OUTP), dtype=f16)
    w2f[:, :OUT] = W2.astype(f16)
    b2f = np.zeros((P, OUTP), dtype=f16)
    b2f[:, :OUT] = np.asarray(b2, dtype=f16).reshape(1, OUT)
    iota_np = np.ascontiguousarray(
        np.broadcast_to(
            np.tile(np.arange(P, dtype=f16), CHMAX).reshape(1, CHMAX * P),
            (P, CHMAX * P),
        )
    )

    for c in range(M):
        # x tile layout: xth[p, (nt*KT+k)*P + n] = x[c*NP + nt*P + n, k*P + p]
        xt = np.zeros((IN, NPAD), dtype=f16)
        xt[:, :NP] = x[c * NP : (c + 1) * NP].T
        xth = np.ascontiguousarray(
            xt.reshape(KT, P, NT, P).transpose(1, 2, 0, 3).reshape(P, NT * KT * P)
        )
        # Pad tokens fetch row 0 (valid data, killed by the drel=255 mask):
        # num_idxs_reg must equal the count of non-negative indices (the
        # decode ring-space reservation and the ucode's trailing-negative
        # trim must stay in lockstep), so keep every slot valid.
        idx16 = np.zeros(TOK, dtype=np.int16)
        drel = np.full(TOK, 255.0, dtype=np.float16)
        for w in range(NT):
            for h in range(NH):
                gi = (c * NT + w) * NH + h
                s0, s1 = group_starts[gi], group_starts[gi + 1]
                k = s1 - s0
                base = slot_base[w, h]
                idx16[base : base + k] = lidx_s[s0:s1]
                drel[base : base + k] = drel_s[s0:s1]
        idx_w = np.tile(idx16.reshape(-1, 16).T, (8, 1))
        grel = np.ascontiguousarray(drel.reshape(-1, P).T)
        dloc = np.ones(NPAD, np.float32)
        dloc[:NP] = dis[c * NP : (c + 1) * NP]
        disS = np.ascontiguousarray(dloc.reshape(NT, P).T)
        invdC = np.ascontiguousarray((1.0 / dloc).astype(np.float32).reshape(NT, P).T)
        in_maps.append(
            {
                "xth": xth,
                "w1": w1f,
                "b1": b1f,
                "w2": w2f,
                "b2": b2f,
                "iota": np.array(iota_np),
                "idx": np.ascontiguousarray(idx_w),
                "grel": grel,
                "disS": disS,
                "invdC": invdC,
            }
        )

    meta = dict(
        N=N, IN=IN, HID=HID, OUT=OUT, OUTP=OUTP, NP=NP, NPAD=NPAD, NT=NT,
        KT=KT, CHMAX=CHMAX, TOK=TOK, NCHUNK=NCHUNK,
        CH_wh=[[int(CH_wh[w, h]) for h in range(NH)] for w in range(NT)],
        CNT_wh=[[int(counts.max(axis=0)[w, h]) for h in range(NH)]
                for w in range(NT)],
        col_of=[[int(col_of[w, h]) for h in range(NH)] for w in range(NT)],
        slot_base=[[int(slot_base[w, h]) for h in range(NH)] for w in range(NT)],
        piece_rows=piece_rows,
        bound=[int(b) for b in bound],
    )
    return in_maps, meta


def _build(meta):
    import os

    import concourse.mybir as mybir
    import concourse.tile as tile
    from concourse import bacc
    from concourse.bass import ts
    from concourse.masks import make_identity

    IN, HID, OUT, OUTP = meta["IN"], meta["HID"], meta["OUT"], meta["OUTP"]
    NPAD, NT, KT = meta["NPAD"], meta["NT"], meta["KT"]
    CHMAX, TOK, NCHUNK = meta["CHMAX"], meta["TOK"], meta["NCHUNK"]
    CH_wh = meta["CH_wh"]
    CNT_wh = meta["CNT_wh"]
    col_of = meta["col_of"]
    slot_base = meta["slot_base"]
    piece_rows = meta["piece_rows"]
    bound = meta["bound"]
    HC = HID // P
    f16 = mybir.dt.float16
    f32 = mybir.dt.float32

    NQ = 4
    SP = os.environ.get("GCN_SP", "1") == "1"
    MSGB1 = int(os.environ.get("GCN_MSGB1", "8"))
    MSGB2 = int(os.environ.get("GCN_MSGB2", "8"))
    nc = bacc.Bacc(
        "TRN2",
        target_bir_lowering=False,
        debug=False,
        num_devices=M,
        num_swdge_queues=NQ,
    )

    xth_d = nc.dram_tensor("xth", [P, NT * KT * P], f16, kind="ExternalInput")
    w1_d = nc.dram_tensor("w1", [IN, HID], f16, kind="ExternalInput")
    b1_d = nc.dram_tensor("b1", [P, HID], f16, kind="ExternalInput")
    w2_d = nc.dram_tensor("w2", [HID, OUTP], f16, kind="ExternalInput")
    b2_d = nc.dram_tensor("b2", [P, OUTP], f16, kind="ExternalInput")
    iota_d = nc.dram_tensor("iota", [P, CHMAX * P], f16, kind="ExternalInput")
    idx_d = nc.dram_tensor("idx", [P, TOK // 16], mybir.dt.int16, kind="ExternalInput")
    grel_d = nc.dram_tensor("grel", [P, NCHUNK], f16, kind="ExternalInput")
    disS_d = nc.dram_tensor("disS", [P, NT], f32, kind="ExternalInput")
    invdC_d = nc.dram_tensor("invdC", [P, NT], f32, kind="ExternalInput")
    out_d = nc.dram_tensor("out", [NPAD, OUT], f32, kind="ExternalOutput")

    h1_loc = [
        nc.dram_tensor(f"h1_loc{j}", [piece_rows[j], HID], f16) for j in range(NH)
    ]
    h2_loc = [
        nc.dram_tensor(f"h2_loc{j}", [piece_rows[j], OUTP], f16) for j in range(NH)
    ]
    h1_gl = [
        nc.dram_tensor(f"h1_gl{j}", [piece_rows[j] * M, HID], f16,
                       addr_space="Shared")
        for j in range(NH)
    ]
    h2_gl = [
        nc.dram_tensor(f"h2_gl{j}", [piece_rows[j] * M, OUTP], f16,
                       addr_space="Shared")
        for j in range(NH)
    ]

    rg = [list(range(M))]

    def win_piece(w):
        for j in range(NH):
            if bound[j] <= w < bound[j + 1]:
                return j, w - bound[j]
        raise AssertionError(w)

    qctr = [0]

    with tile.TileContext(nc) as tc:
        with (
            tc.tile_pool(name="const", bufs=1) as cp,
            tc.tile_pool(name="work", bufs=3) as wp,
            tc.tile_pool(name="gpool", bufs=5) as gp,
            tc.tile_pool(name="msg1", bufs=MSGB1) as mp1,
            tc.tile_pool(name="msg2", bufs=MSGB2) as mp2,
            tc.tile_pool(name="psum", bufs=3, space="PSUM") as pp,
        ):
            # ---- constants ----
            w1t = cp.tile([P, KT, HID], f16)
            nc.sync.dma_start(
                out=w1t[:], in_=w1_d[:, :].rearrange("(k p) h -> p k h", p=P)
            )
            w2t = cp.tile([P, HC, OUTP], f16)
            nc.sync.dma_start(
                out=w2t[:], in_=w2_d[:, :].rearrange("(k p) o -> p k o", p=P)
            )
            iota_t = cp.tile([P, CHMAX * P], f16)
            nc.sync.dma_start(out=iota_t[:], in_=iota_d[:, :])
            ident = cp.tile([P, P], f16)
            make_identity(nc, ident[:])
            b1s = cp.tile([P, HID], f16)
            nc.sync.dma_start(out=b1s[:], in_=b1_d[:, :])
            b2s = cp.tile([P, OUTP], f16)
            nc.sync.dma_start(out=b2s[:], in_=b2_d[:, :])
            grelS = cp.tile([P, NCHUNK], f16)
            nc.sync.dma_start(out=grelS[:], in_=grel_d[:, :])
            disS = cp.tile([P, NT], f32)
            nc.sync.dma_start(out=disS[:], in_=disS_d[:, :])
            invdC = cp.tile([P, NT], f32)
            nc.sync.dma_start(out=invdC[:], in_=invdC_d[:, :])
            idxS = cp.tile([P, TOK // 16], mybir.dt.int16)
            nc.sync.dma_start(out=idxS[:], in_=idx_d[:, :])
            # node tables and inter-pass spills stay resident in SBUF
            h1all = cp.tile([P, NT, HID], f16)
            h2all = cp.tile([P, NT, OUTP], f16)
            sp1 = cp.tile([P, NT, HID], f16)
            sp2 = cp.tile([P, NT, OUTP], f16)

            # ---- stage 1: h1' = dis * (x @ W1), AllGather in NH pieces ----
            for w in range(NT):
                j, wo = win_piece(w)
                xtt = wp.tile([P, KT, P], f16, tag="xtt")
                nc.sync.dma_start(
                    out=xtt[:],
                    in_=xth_d[:, w * KT * P : (w + 1) * KT * P].rearrange(
                        "p (k n) -> p k n", n=P
                    ),
                )
                ph = pp.tile([P, HID], f32, tag="acc256")
                for k in range(KT):
                    nc.tensor.matmul(
                        ph[:],
                        lhsT=xtt[:, k, :],
                        rhs=w1t[:, k, :],
                        start=(k == 0),
                        stop=(k == KT - 1),
                    )
                nc.scalar.activation(
                    h1all[:, w, :], ph[:], mybir.ActivationFunctionType.Copy,
                    scale=disS[:, w : w + 1],
                )
                nc.sync.dma_start(out=h1_loc[j][ts(wo, P), :], in_=h1all[:, w, :])
                if w == bound[j + 1] - 1:
                    nc.gpsimd.collective_compute(
                        "AllGather",
                        mybir.AluOpType.bypass,
                        replica_groups=rg,
                        ins=[h1_loc[j].ap().opt()],
                        outs=[h1_gl[j].ap().opt()],
                    )

            def build_GW(w, h):
                chw = CH_wh[w][h]
                c0 = col_of[w][h]
                GW = gp.tile([P, CHMAX * P], f16, tag="GW")
                nc.vector.tensor_tensor(
                    out=GW[:, : chw * P].rearrange("p (c e) -> p c e", e=P),
                    in0=iota_t[:, : chw * P].rearrange("p (c e) -> p c e", e=P),
                    in1=grelS[:, c0 : c0 + chw].to_broadcast([P, chw, P]),
                    op=mybir.AluOpType.is_equal,
                )
                return GW

            def gathers(w, h, table, elem, pool, mtag):
                """Issue dma_gather calls for (w, h); returns msg tile."""
                chw = CH_wh[w][h]
                if chw == 0:
                    return None
                mt = pool.tile([P, CHMAX, elem], f16, tag=mtag)
                base = slot_base[w][h]
                for s in range(0, chw * P, GCAP * P):
                    gn = min(GCAP * P, chw * P - s)
                    nc.gpsimd.dma_gather(
                        out_ap=mt[:, s // P : (s + gn) // P, :],
                        in_ap=table[h][:, :],
                        idxs_ap=idxS[:, (base + s) // 16 : (base + s + gn) // 16],
                        num_idxs=gn,
                        num_idxs_reg=gn,
                        elem_size=elem,
                        queue_num=qctr[0] % NQ,
                        single_packet=SP,
                    )
                    qctr[0] += 1
                return mt

            def accum_pass(w, p, mt, acc, rhs0, elem):
                """identity(rhs0) matmul + this pass's edge-chunk matmuls."""
                chw = CH_wh[w][p] if mt is not None else 0
                GW = build_GW(w, p) if chw else None
                nc.tensor.matmul(
                    acc[:], lhsT=ident[:], rhs=rhs0,
                    start=True, stop=(chw == 0),
                )
                for c in range(chw):
                    nc.tensor.matmul(
                        acc[:],
                        lhsT=GW[:, ts(c, P)],
                        rhs=mt[:, c, :],
                        start=False,
                        stop=(c == chw - 1),
                    )

            # ---- stages 3+5: pass-based aggregation ----
            deferred_fire = []

            def agg_layer(table, elem, pool, mtag, bvec, hall, spill, finish,
                          fire_piece):
                for pss in range(NH):
                    pending_fire = []
                    for w in range(NT):
                        # fire AGs deferred from the previous layer once the
                        # gather pipeline is warmed up (keeps GpSimd FIFO hot)
                        if (pss == 0 and w == min(AG2_SKEW, NT - 1)
                                and fire_piece is None):
                            while deferred_fire:
                                deferred_fire.pop(0)()
                        mt = gathers(w, pss, table, elem, pool, mtag)
                        acc = pp.tile([P, elem], f32,
                                      tag="acc256" if elem == HID else "acc128")
                        if pss == 0:
                            rhs0 = wp.tile([P, elem], f16, tag=f"ownb{elem}")
                            # rhs0 = bvec * (1/dis[d]) + own-row (self loop)
                            nc.vector.scalar_tensor_tensor(
                                out=rhs0[:],
                                in0=bvec[:],
                                scalar=invdC[:, w : w + 1],
                                in1=hall[:, w, :],
                                op0=mybir.AluOpType.mult,
                                op1=mybir.AluOpType.add,
                            )
                            rhs0 = rhs0[:]
                        else:
                            rhs0 = spill[:, w, :]
                        accum_pass(w, pss, mt, acc, rhs0, elem)
                        if pss < NH - 1:
                            nc.scalar.activation(
                                spill[:, w, :], acc[:],
                                mybir.ActivationFunctionType.Copy,
                            )
                        else:
                            finish(w, acc)
                            j, wo = win_piece(w)
                            if fire_piece is not None and w == bound[j + 1] - 1:
                                pending_fire.append((j, w + AG2_SKEW))
                        # fire queued piece-AGs once gathers have moved past
                        for (j, at) in list(pending_fire):
                            if w >= at:
                                fire_piece(j)
                                pending_fire.remove((j, at))
                    # pieces whose skew point lies beyond the loop: piece j's
                    # table is first read by the next layer's pass-j gathers,
                    # so j>=1 can defer into the next layer's pass-0 stream
                    # (avoids a FIFO stall on the last windows' PSUM drain);
                    # piece 0 must fire now.
                    for (j, at) in pending_fire:
                        if j == 0:
                            fire_piece(j)
                        else:
                            deferred_fire.append(lambda j=j: fire_piece(j))

            # layer 1 finish: relu, scale, z1 @ W2, store h2
            def finish_l1(w, pz):
                j, wo = win_piece(w)
                z1r = wp.tile([P, HID], f16, tag="z1r")
                nc.scalar.activation(
                    z1r[:], pz[:], mybir.ActivationFunctionType.Relu,
                    scale=disS[:, w : w + 1],
                )
                ph2 = pp.tile([P, OUTP], f32, tag="acc128")
                for k in range(HC):
                    pt = pp.tile([P, P], f16, tag="acct", bufs=2)
                    nc.tensor.transpose(pt[:], z1r[:, ts(k, P)], ident[:])
                    zt = wp.tile([P, P], f16, tag="zt")
                    nc.vector.tensor_copy(zt[:], pt[:])
                    nc.tensor.matmul(
                        ph2[:],
                        lhsT=zt[:],
                        rhs=w2t[:, k, :],
                        start=(k == 0),
                        stop=(k == HC - 1),
                    )
                nc.scalar.activation(
                    h2all[:, w, :], ph2[:], mybir.ActivationFunctionType.Copy,
                    scale=disS[:, w : w + 1],
                )
                nc.sync.dma_start(out=h2_loc[j][ts(wo, P), :], in_=h2all[:, w, :])

            def fire_ag2(j):
                nc.gpsimd.collective_compute(
                    "AllGather",
                    mybir.AluOpType.bypass,
                    replica_groups=rg,
                    ins=[h2_loc[j].ap().opt()],
                    outs=[h2_gl[j].ap().opt()],
                )

            agg_layer(h1_gl, HID, mp1, "m1", b1s, h1all, sp1, finish_l1,
                      fire_ag2)

            # layer 2 finish: scale, store out
            def finish_l2(w, po):
                os_ = wp.tile([P, OUT], f32, tag="os")
                nc.scalar.activation(
                    os_[:], po[:, :OUT], mybir.ActivationFunctionType.Copy,
                    scale=disS[:, w : w + 1],
                )
                nc.sync.dma_start(out=out_d[ts(w, P), :], in_=os_[:])

            agg_layer(h2_gl, OUTP, mp2, "m2", b2s, h2all, sp2, finish_l2, None)

    nc.compile()
    return nc


def kernel(x, W1, b1, W2, b2, edge_index, _run_opts=None):
    from concourse.bass_utils import run_bass_kernel_spmd

    x = np.asarray(x)
    edge_index = np.asarray(edge_index)
    in_maps, meta = _prep(
        x, np.asarray(W1), np.asarray(b1), np.asarray(W2), np.asarray(b2), edge_index
    )
    nc = _build(meta)
    opts = dict(_run_opts or {})
    opts.pop("_bass_results", None)
    res = run_bass_kernel_spmd(nc, in_maps, core_ids=list(range(M)), **opts)
    NP, OUT = meta["NP"], meta["OUT"]
    out = np.concatenate(
        [res.results[c]["out"][:NP] for c in range(M)], axis=0
    ).astype(np.float32)
    if _run_opts is not None:
        _run_opts["_bass_results"] = res
    return out
